# revision 1
# baseline (speedup 1.0000x reference)
"""LoRA layer kernel for Trainium2, 8-core data-parallel.

out = x @ W.T + 2.0 * ((x @ B) @ A)
  x: (4, 4096, 4096) f32, W: (4096, 4096), A: (16, 4096), B: (4096, 16)

Strategy: flatten x to (16384, 4096) rows, shard rows across 8 cores
(2048 rows each), replicate W/A/B. Per core a single fused GEMM:
  - x-block stationary (fp32r), W.T streamed as moving operand
  - LoRA: tT = (x @ B).T computed per block (contraction over full K),
    then one extra K=16 matmul per (m_tile, o_chunk) accumulates
    2*(x@B)@A into the same PSUM bank (A pre-scaled by 2 on host).
All device matmuls use float32r: 1 cycle/row at N=512 (same rate as
bf16, ~TF32 precision).
"""

import sys

if "/opt/trn_rl_repo" not in sys.path:
    sys.path.insert(0, "/opt/trn_rl_repo")

import os

import numpy as np

import concourse.bass as bass
import concourse.mybir as mybir
import concourse.tile as tile

N_CORES = 8
D = 4096
RANK = 16
ROWS_TOTAL = 4 * 4096          # 16384
ROWS_PER_CORE = ROWS_TOTAL // N_CORES  # 2048
P = 128
KT = D // P                    # 32 k-tiles
M_BLOCK = 1024                 # rows per x-resident block
N_BLOCKS = ROWS_PER_CORE // M_BLOCK    # 2
MT_PER_BLOCK = M_BLOCK // P    # 8 m-tiles (PSUM banks)
OC = 512                       # o-chunk width (one PSUM bank)
N_OC = D // OC                 # 8
KH = KT // 2                   # k-tiles per x half-tile

F32 = mybir.dt.float32
F32R = mybir.dt.float32r

W_PAIR = os.environ.get("K_WPAIR", "1") == "1"
GP_DMA = os.environ.get("K_GPDMA", "1") == "1"
WARMUP = os.environ.get("K_WARMUP", "1") == "1"


def _dma_gp(nc):
    return nc.gpsimd if GP_DMA else nc.sync


def split_wide_waits(nc, max_waits=1):
    """walrus in this container rejects >1 sync wait per instruction;
    move excess waits onto preceding same-engine NoOps."""
    n_split = 0
    for f in nc.m.functions:
        for bb in f.blocks:
            new_insts = []
            for inst in bb.instructions:
                si = getattr(inst, "sync_info", None)
                if si is not None and si.on_wait and len(si.on_wait) > max_waits:
                    waits = list(si.on_wait)
                    keep = waits[-max_waits:]
                    extra = waits[:-max_waits]
                    for i in range(0, len(extra), max_waits):
                        chunk = extra[i:i + max_waits]
                        nop = mybir.InstNoOp(
                            name=f"{inst.name}_wsplit{i}",
                            sync_info=mybir.SyncInfo(on_wait=chunk, on_update=[]),
                            bass_nofuse=True,
                            engine=inst.engine,
                        )
                        new_insts.append(nop)
                        n_split += 1
                    si.on_wait = keep
                new_insts.append(inst)
            bb.instructions[:] = new_insts
    return n_split


def build_program():
    nc = bass.Bass()
    xt = nc.declare_dram_parameter("xt", [D, ROWS_PER_CORE], F32R, isOutput=False)
    wt = nc.declare_dram_parameter("wt", [D, D], F32R, isOutput=False)
    # bmat pre-arranged on host: [128, KT*RANK], col-block k = rows k*128..+128
    bmat = nc.declare_dram_parameter("bmat", [P, KT * RANK], F32R, isOutput=False)
    a2 = nc.declare_dram_parameter("a2", [RANK, D], F32R, isOutput=False)
    out = nc.declare_dram_parameter("out", [ROWS_PER_CORE, D], F32, isOutput=True)

    with tile.TileContext(nc) as tc:
        with (
            tc.tile_pool(name="xpool_a", bufs=1) as xpool_a,
            tc.tile_pool(name="xpool_b", bufs=1) as xpool_b,
            tc.tile_pool(name="wpool", bufs=6) as wpool,
            tc.tile_pool(name="opool", bufs=4) as opool,
            tc.tile_pool(name="cpool", bufs=1) as cpool,
            tc.tile_pool(name="tpool", bufs=2) as tpool,
            tc.tile_pool(name="ppool", bufs=8, space="PSUM") as ppool,
        ):
            # constants: B (pre-arranged) and A2 — single DMAs on gpsimd queue
            btile = cpool.tile([P, KT * RANK], F32R, tag="bt")
            _dma_gp(nc).dma_start(btile[:], bmat[:])
            atile = cpool.tile([RANK, D], F32R, tag="at")
            _dma_gp(nc).dma_start(atile[:], a2[:])

            # HAM warmup: ~5us of dummy matmuls so the PE clock is at 8/8
            # before real work lands (3.4us busy window un-throttles).
            if WARMUP:
                junk = ppool.tile([RANK, OC], F32, tag="acc", name="junk")
                for i in range(25):
                    nc.tensor.matmul(
                        junk[:],
                        btile[:, :RANK],
                        btile[:, :OC],
                        start=(i == 0),
                        stop=(i == 24),
                    )

            for blk in range(N_BLOCKS):
                r0 = blk * M_BLOCK
                # x block resident: two half tiles (k 0-15, k 16-31)
                xa = xpool_a.tile([P, KH * M_BLOCK], F32R, tag="xa")
                xb = xpool_b.tile([P, KH * M_BLOCK], F32R, tag="xb")

                def xsl(k, c0, cw):
                    t = xa if k < KH else xb
                    kk = k % KH
                    return t[:, kk * M_BLOCK + c0: kk * M_BLOCK + c0 + cw]

                for k in range(KT):
                    eng = (nc.gpsimd if k % 2 == 0 else nc.scalar) if GP_DMA else nc.sync
                    eng.dma_start(
                        xsl(k, 0, M_BLOCK),
                        xt[k * P:(k + 1) * P, r0:r0 + M_BLOCK],
                    )

                # stage A: tT[r, m] = sum_i B[i,r] * x[m,i]  (per block)
                tT = tpool.tile([RANK, M_BLOCK], F32R, tag="tT")
                for h in range(M_BLOCK // OC):
                    pt = ppool.tile([RANK, OC], F32, tag="acc")
                    for k in range(KT):
                        nc.tensor.matmul(
                            pt[:],
                            btile[:, k * RANK:(k + 1) * RANK],
                            xsl(k, h * OC, OC),
                            start=(k == 0),
                            stop=(k == KT - 1),
                        )
                    nc.vector.tensor_copy(tT[:, h * OC:(h + 1) * OC], pt[:])

                # main GEMM + fused LoRA accumulation.
                # W fetched as adjacent k-tile pairs [128, 2*OC] (halves the
                # ~0.6us/DMA issue count on the sync queue).
                for oc in range(N_OC):
                    psums = []
                    for mt in range(MT_PER_BLOCK):
                        psums.append(ppool.tile([P, OC], F32, tag="acc", name=f"ps_{blk}_{oc}_{mt}"))
                    for k2 in range(KT // 2):
                        wtile = wpool.tile([P, 2 * OC], F32R, tag="wt")
                        src = wt[k2 * 2 * P:(k2 + 1) * 2 * P,
                                 oc * OC:(oc + 1) * OC]
                        if W_PAIR:
                            nc.sync.dma_start(
                                wtile.rearrange("p (b c) -> p b c", b=2),
                                src.rearrange("(b p) c -> p b c", p=P),
                            )
                        else:
                            for half in range(2):
                                nc.sync.dma_start(
                                    wtile[:, half * OC:(half + 1) * OC],
                                    wt[(2 * k2 + half) * P:(2 * k2 + half + 1) * P,
                                       oc * OC:(oc + 1) * OC],
                                )
                        for half in range(2):
                            k = 2 * k2 + half
                            for mt in range(MT_PER_BLOCK):
                                nc.tensor.matmul(
                                    psums[mt][:],
                                    xsl(k, mt * P, P),
                                    wtile[:, half * OC:(half + 1) * OC],
                                    start=(k == 0),
                                    stop=False,
                                )
                    for mt in range(MT_PER_BLOCK):
                        # LoRA: += tT[:, mt].T @ (2A[:, oc])
                        nc.tensor.matmul(
                            psums[mt][:],
                            tT[:, mt * P:(mt + 1) * P],
                            atile[:, oc * OC:(oc + 1) * OC],
                            start=False,
                            stop=True,
                        )
                        ot = opool.tile([P, OC], F32, tag="ot")
                        nc.vector.tensor_copy(ot[:], psums[mt][:])
                        nc.sync.dma_start(
                            out[r0 + mt * P:r0 + (mt + 1) * P,
                                oc * OC:(oc + 1) * OC],
                            ot[:],
                        )

    split_wide_waits(nc)
    return nc


_NC_CACHE = [None]


def kernel(x, weight, lora_A, lora_B):
    from concourse.bass_utils import run_bass_kernel_spmd

    x = np.asarray(x, dtype=np.float32)
    weight = np.asarray(weight, dtype=np.float32)
    lora_A = np.asarray(lora_A, dtype=np.float32)
    lora_B = np.asarray(lora_B, dtype=np.float32)

    x2 = x.reshape(ROWS_TOTAL, D)
    wt = np.ascontiguousarray(weight.T)
    a2 = np.ascontiguousarray(2.0 * lora_A)
    # pre-arrange B: [128, KT*RANK], col-block k holds rows k*128..(k+1)*128
    bmat = np.ascontiguousarray(
        lora_B.reshape(KT, P, RANK).transpose(1, 0, 2).reshape(P, KT * RANK)
    )

    in_maps = []
    for c in range(N_CORES):
        xt_c = np.ascontiguousarray(
            x2[c * ROWS_PER_CORE:(c + 1) * ROWS_PER_CORE].T
        )
        in_maps.append({"xt": xt_c, "wt": wt, "bmat": bmat, "a2": a2})

    if _NC_CACHE[0] is None:
        _NC_CACHE[0] = build_program()
    nc = _NC_CACHE[0]

    res = run_bass_kernel_spmd(nc, in_maps, list(range(N_CORES)))
    out = np.concatenate(
        [res.results[c]["out"] for c in range(N_CORES)], axis=0
    )
    return out.reshape(x.shape)



# revision 2
# speedup vs baseline: 1.1697x; 1.1697x over previous
"""LoRA layer kernel for Trainium2, 8-core data-parallel.

out = x @ W.T + 2.0 * ((x @ B) @ A)
  x: (4, 4096, 4096) f32, W: (4096, 4096), A: (16, 4096), B: (4096, 16)

v2 strategy:
  - Host folds LoRA into the weight: W' = W.T + 2*(B@A)  ([in,out]),
    so the device runs a single pure GEMM out = x @ W'.
  - bf16 operands (same PE rate as fp32r at 1 col/cycle, half the DMA
    traffic; fp32 PSUM accumulation keeps rel err ~3e-3).
  - Rows sharded across 8 cores (2048 rows each). Per core the x panel
    ([4096 K, 2048 M] bf16 = 128 KB/partition) is fully SBUF-resident,
    so W' streams from HBM exactly once (32 MB bf16).
  - W-stationary / x-moving: out.T[o,m] = sum_k W'[k,o]*x[k,m]. Each
    pass covers a 256-wide o-pair: 8 PSUM banks = 2 o-tiles x 4
    m-chunks, accumulated over all 32 k-tiles (K-contiguous keeps the
    PE HAM-warm). 4 consecutive matmuls share one stationary W tile.
  - Output produced transposed ([4096 o, 2048 m] per core); host
    transposes back when gathering.
"""

import sys

if "/opt/trn_rl_repo" not in sys.path:
    sys.path.insert(0, "/opt/trn_rl_repo")

import numpy as np
import ml_dtypes

import concourse.bass as bass
import concourse.mybir as mybir
import concourse.tile as tile

N_CORES = 8
D = 4096
RANK = 16
ROWS_TOTAL = 4 * 4096                   # 16384
ROWS_PER_CORE = ROWS_TOTAL // N_CORES   # 2048
P = 128
NKT = D // P                            # 32 k-tiles
NPASS = 16                              # o-pairs of 256
OPW = 256                               # o columns per pass
MC = 512                                # moving m-chunk width
NMC = ROWS_PER_CORE // MC               # 4 m-chunks
XCH = 2                                 # k-tiles per x DMA chunk
NXCH = NKT // XCH                       # 16 x chunks

F32 = mybir.dt.float32
BF16 = mybir.dt.bfloat16
BF = ml_dtypes.bfloat16


def split_wide_waits(nc, max_waits=1):
    """walrus in this container rejects >1 sync wait per instruction;
    move excess waits onto preceding same-engine NoOps."""
    n_split = 0
    for f in nc.m.functions:
        for bb in f.blocks:
            new_insts = []
            for inst in bb.instructions:
                si = getattr(inst, "sync_info", None)
                if si is not None and si.on_wait and len(si.on_wait) > max_waits:
                    waits = list(si.on_wait)
                    keep = waits[-max_waits:]
                    extra = waits[:-max_waits]
                    for i in range(0, len(extra), max_waits):
                        chunk = extra[i:i + max_waits]
                        nop = mybir.InstNoOp(
                            name=f"{inst.name}_wsplit{i}",
                            sync_info=mybir.SyncInfo(on_wait=chunk, on_update=[]),
                            bass_nofuse=True,
                            engine=inst.engine,
                        )
                        new_insts.append(nop)
                        n_split += 1
                    si.on_wait = keep
                new_insts.append(inst)
            bb.instructions[:] = new_insts
    return n_split


def build_program():
    nc = bass.Bass()
    # x panel, host pre-arranged: [128 part, 32 ktile * 2048 m] bf16
    xp = nc.declare_dram_parameter("xp", [P, NKT * ROWS_PER_CORE], BF16, isOutput=False)
    # W' panel stream, host pre-arranged: [128 part, 16 pass * 32 kt * 256 o]
    wp = nc.declare_dram_parameter("wp", [P, NPASS * NKT * OPW], BF16, isOutput=False)
    outT = nc.declare_dram_parameter("outT", [D, ROWS_PER_CORE], F32, isOutput=True)

    PASS_W = NKT * OPW          # 8192 cols per pass panel
    HALF_W = PASS_W // 2        # 4096 cols per half panel (16 k-tiles)

    with tile.TileContext(nc) as tc:
        with (
            tc.tile_pool(name="xpool", bufs=1) as xpool,
            tc.tile_pool(name="wpool", bufs=2) as wpool,
            tc.tile_pool(name="opool", bufs=8) as opool,
            tc.tile_pool(name="ppool", bufs=8, space="PSUM") as ppool,
        ):
            # resident x: 16 chunks of 2 k-tiles, alternating DMA queues
            xts = []
            for i in range(NXCH):
                xt = xpool.tile([P, XCH * ROWS_PER_CORE], BF16, tag=f"x{i}",
                                name=f"x{i}")
                eng = nc.gpsimd if i % 2 == 0 else nc.scalar
                eng.dma_start(
                    xt[:],
                    xp[:, i * XCH * ROWS_PER_CORE:(i + 1) * XCH * ROWS_PER_CORE],
                )
                xts.append(xt)

            def xsl(k, mc):
                t = xts[k // XCH]
                off = (k % XCH) * ROWS_PER_CORE + mc * MC
                return t[:, off:off + MC]

            for p in range(NPASS):
                # W half-panels (16 k-tiles x 256 o each), double-buffered
                wA = wpool.tile([P, HALF_W], BF16, tag="wA", name=f"wA{p}")
                nc.sync.dma_start(
                    wA[:], wp[:, p * PASS_W:p * PASS_W + HALF_W])
                wB = wpool.tile([P, HALF_W], BF16, tag="wB", name=f"wB{p}")
                nc.sync.dma_start(
                    wB[:], wp[:, p * PASS_W + HALF_W:(p + 1) * PASS_W])

                psums = []
                for ot in range(2):
                    for mcn in range(NMC):
                        psums.append(ppool.tile(
                            [P, MC], F32, tag="acc", name=f"ps_{p}_{ot}_{mcn}"))

                for k in range(NKT):
                    wt = wA if k < 16 else wB
                    koff = (k % 16) * OPW
                    for ot in range(2):
                        stat = wt[:, koff + ot * P:koff + (ot + 1) * P]
                        for mcn in range(NMC):
                            nc.tensor.matmul(
                                psums[ot * NMC + mcn][:],
                                stat,
                                xsl(k, mcn),
                                start=(k == 0),
                                stop=(k == NKT - 1),
                            )

                for idx, ps in enumerate(psums):
                    ot, mcn = divmod(idx, NMC)
                    o0 = p * OPW + ot * P
                    otile = opool.tile([P, MC], F32, tag="ot", name=f"o_{p}_{idx}")
                    nc.vector.tensor_copy(otile[:], ps[:])
                    nc.scalar.dma_start(
                        outT[o0:o0 + P, mcn * MC:(mcn + 1) * MC],
                        otile[:],
                    )

    split_wide_waits(nc)
    return nc


_NC_CACHE = [None]


def kernel(x, weight, lora_A, lora_B):
    from concourse.bass_utils import run_bass_kernel_spmd

    x = np.asarray(x, dtype=np.float32)
    weight = np.asarray(weight, dtype=np.float32)
    lora_A = np.asarray(lora_A, dtype=np.float32)
    lora_B = np.asarray(lora_B, dtype=np.float32)

    # fold LoRA: out = x @ (W.T + 2*B@A)
    W2 = (weight.T + 2.0 * (lora_B @ lora_A)).astype(BF)
    # [k, o] -> [part, pass, kt, opw] -> flat [128, NPASS*NKT*OPW]
    w4 = np.ascontiguousarray(
        W2.reshape(NKT, P, NPASS, OPW).transpose(1, 2, 0, 3)
    ).reshape(P, NPASS * NKT * OPW)

    x2 = x.reshape(ROWS_TOTAL, D).astype(BF)

    in_maps = []
    for c in range(N_CORES):
        xc = x2[c * ROWS_PER_CORE:(c + 1) * ROWS_PER_CORE]      # [2048 m, 4096 k]
        # [k, m] -> [part, kt, m] -> flat [128, NKT*2048]
        x3 = np.ascontiguousarray(
            xc.T.reshape(NKT, P, ROWS_PER_CORE).transpose(1, 0, 2)
        ).reshape(P, NKT * ROWS_PER_CORE)
        in_maps.append({"xp": x3, "wp": w4})

    if _NC_CACHE[0] is None:
        _NC_CACHE[0] = build_program()
    nc = _NC_CACHE[0]

    res = run_bass_kernel_spmd(nc, in_maps, list(range(N_CORES)))
    out = np.empty((ROWS_TOTAL, D), dtype=np.float32)
    for c in range(N_CORES):
        out[c * ROWS_PER_CORE:(c + 1) * ROWS_PER_CORE] = res.results[c]["outT"].T
    return out.reshape(x.shape)


# revision 4
# speedup vs baseline: 1.1841x; 1.0123x over previous
"""LoRA layer kernel for Trainium2, 8-core data-parallel.

out = x @ W.T + 2.0 * ((x @ B) @ A)
  x: (4, 4096, 4096) f32, W: (4096, 4096), A: (16, 4096), B: (4096, 16)

v2 strategy:
  - Host folds LoRA into the weight: W' = W.T + 2*(B@A)  ([in,out]),
    so the device runs a single pure GEMM out = x @ W'.
  - bf16 operands (same PE rate as fp32r at 1 col/cycle, half the DMA
    traffic; fp32 PSUM accumulation keeps rel err ~3e-3).
  - Rows sharded across 8 cores (2048 rows each). Per core the x panel
    ([4096 K, 2048 M] bf16 = 128 KB/partition) is fully SBUF-resident,
    so W' streams from HBM exactly once (32 MB bf16).
  - W-stationary / x-moving: out.T[o,m] = sum_k W'[k,o]*x[k,m]. Each
    pass covers a 256-wide o-pair: 8 PSUM banks = 2 o-tiles x 4
    m-chunks, accumulated over all 32 k-tiles (K-contiguous keeps the
    PE HAM-warm). 4 consecutive matmuls share one stationary W tile.
  - Output produced transposed ([4096 o, 2048 m] per core); host
    transposes back when gathering.
"""

import sys

if "/opt/trn_rl_repo" not in sys.path:
    sys.path.insert(0, "/opt/trn_rl_repo")

import numpy as np
import ml_dtypes

import concourse.bass as bass
import concourse.mybir as mybir
import concourse.tile as tile

N_CORES = 8
D = 4096
RANK = 16
ROWS_TOTAL = 4 * 4096                   # 16384
ROWS_PER_CORE = ROWS_TOTAL // N_CORES   # 2048
P = 128
NKT = D // P                            # 32 k-tiles
NPASS = 16                              # o-pairs of 256
OPW = 256                               # o columns per pass
MC = 512                                # moving m-chunk width
NMC = ROWS_PER_CORE // MC               # 4 m-chunks
NWQ = 4                                 # W quarter-panels per pass

F32 = mybir.dt.float32
BF16 = mybir.dt.bfloat16
BF = ml_dtypes.bfloat16


def split_wide_waits(nc, max_waits=1):
    """walrus in this container rejects >1 sync wait per instruction;
    move excess waits onto preceding same-engine NoOps."""
    n_split = 0
    for f in nc.m.functions:
        for bb in f.blocks:
            new_insts = []
            for inst in bb.instructions:
                si = getattr(inst, "sync_info", None)
                if si is not None and si.on_wait and len(si.on_wait) > max_waits:
                    waits = list(si.on_wait)
                    keep = waits[-max_waits:]
                    extra = waits[:-max_waits]
                    for i in range(0, len(extra), max_waits):
                        chunk = extra[i:i + max_waits]
                        nop = mybir.InstNoOp(
                            name=f"{inst.name}_wsplit{i}",
                            sync_info=mybir.SyncInfo(on_wait=chunk, on_update=[]),
                            bass_nofuse=True,
                            engine=inst.engine,
                        )
                        new_insts.append(nop)
                        n_split += 1
                    si.on_wait = keep
                new_insts.append(inst)
            bb.instructions[:] = new_insts
    return n_split


def build_program():
    nc = bass.Bass()
    # x panel, host pre-arranged: [128 part, 32 ktile * 2048 m] bf16
    xp = nc.declare_dram_parameter("xp", [P, NKT * ROWS_PER_CORE], BF16, isOutput=False)
    # W' panel stream, host pre-arranged: [128 part, 16 pass * 32 kt * 256 o]
    wp = nc.declare_dram_parameter("wp", [P, NPASS * NKT * OPW], BF16, isOutput=False)
    outT = nc.declare_dram_parameter("outT", [D, ROWS_PER_CORE], F32, isOutput=True)

    PASS_W = NKT * OPW          # 8192 cols per pass panel
    QW = PASS_W // NWQ          # 2048 cols per quarter panel (8 k-tiles)
    KQ = NKT // NWQ             # 8 k-tiles per quarter panel

    with tile.TileContext(nc) as tc:
        with (
            tc.tile_pool(name="xpool", bufs=1) as xpool,
            tc.tile_pool(name="wpool", bufs=2) as wpool,
            tc.tile_pool(name="opool", bufs=8) as opool,
            tc.tile_pool(name="ppool", bufs=8, space="PSUM") as ppool,
        ):
            # resident x: 32 single-k-tile chunks, one queue, issued in
            # consumption order so the stream pipelines with the k-loop
            xts = []
            for i in range(NKT):
                xt = xpool.tile([P, ROWS_PER_CORE], BF16, tag=f"x{i}",
                                name=f"x{i}")
                nc.gpsimd.dma_start(
                    xt[:],
                    xp[:, i * ROWS_PER_CORE:(i + 1) * ROWS_PER_CORE],
                )
                xts.append(xt)

            def xsl(k, mc):
                return xts[k][:, mc * MC:(mc + 1) * MC]

            for p in range(NPASS):
                # W quarter-panels (8 k-tiles x 256 o each), double-buffered
                wqs = []
                for q in range(NWQ):
                    wq = wpool.tile([P, QW], BF16, tag=f"wq{q}", name=f"w{p}_{q}")
                    nc.sync.dma_start(
                        wq[:], wp[:, p * PASS_W + q * QW:p * PASS_W + (q + 1) * QW])
                    wqs.append(wq)

                psums = []
                for ot in range(2):
                    for mcn in range(NMC):
                        psums.append(ppool.tile(
                            [P, MC], F32, tag="acc", name=f"ps_{p}_{ot}_{mcn}"))

                for k in range(NKT):
                    wt = wqs[k // KQ]
                    koff = (k % KQ) * OPW
                    for ot in range(2):
                        stat = wt[:, koff + ot * P:koff + (ot + 1) * P]
                        for mcn in range(NMC):
                            nc.tensor.matmul(
                                psums[ot * NMC + mcn][:],
                                stat,
                                xsl(k, mcn),
                                start=(k == 0),
                                stop=(k == NKT - 1),
                            )

                # drain: even banks on DVE, odd banks on ACT, out-DMA on Pool
                for idx, ps in enumerate(psums):
                    ot, mcn = divmod(idx, NMC)
                    o0 = p * OPW + ot * P
                    otile = opool.tile([P, MC], F32, tag="ot", name=f"o_{p}_{idx}")
                    if idx % 2 == 0:
                        nc.vector.tensor_copy(otile[:], ps[:])
                    else:
                        nc.scalar.copy(otile[:], ps[:])
                    nc.gpsimd.dma_start(
                        outT[o0:o0 + P, mcn * MC:(mcn + 1) * MC],
                        otile[:],
                    )

    split_wide_waits(nc)
    return nc


_NC_CACHE = [None]


def kernel(x, weight, lora_A, lora_B):
    from concourse.bass_utils import run_bass_kernel_spmd

    x = np.asarray(x, dtype=np.float32)
    weight = np.asarray(weight, dtype=np.float32)
    lora_A = np.asarray(lora_A, dtype=np.float32)
    lora_B = np.asarray(lora_B, dtype=np.float32)

    # fold LoRA: out = x @ (W.T + 2*B@A)
    W2 = (weight.T + 2.0 * (lora_B @ lora_A)).astype(BF)
    # [k, o] -> [part, pass, kt, opw] -> flat [128, NPASS*NKT*OPW]
    w4 = np.ascontiguousarray(
        W2.reshape(NKT, P, NPASS, OPW).transpose(1, 2, 0, 3)
    ).reshape(P, NPASS * NKT * OPW)

    x2 = x.reshape(ROWS_TOTAL, D).astype(BF)

    in_maps = []
    for c in range(N_CORES):
        xc = x2[c * ROWS_PER_CORE:(c + 1) * ROWS_PER_CORE]      # [2048 m, 4096 k]
        # [k, m] -> [part, kt, m] -> flat [128, NKT*2048]
        x3 = np.ascontiguousarray(
            xc.T.reshape(NKT, P, ROWS_PER_CORE).transpose(1, 0, 2)
        ).reshape(P, NKT * ROWS_PER_CORE)
        in_maps.append({"xp": x3, "wp": w4})

    if _NC_CACHE[0] is None:
        _NC_CACHE[0] = build_program()
    nc = _NC_CACHE[0]

    res = run_bass_kernel_spmd(nc, in_maps, list(range(N_CORES)))
    out = np.empty((ROWS_TOTAL, D), dtype=np.float32)
    for c in range(N_CORES):
        out[c * ROWS_PER_CORE:(c + 1) * ROWS_PER_CORE] = res.results[c]["outT"].T
    return out.reshape(x.shape)
